# revision 39
# baseline (speedup 1.0000x reference)
"""Causal multi-head attention Trainium2 kernel (8 NeuronCores).

Problem: B=4, L=2048, D=1024, 16 heads x (dh=64, dv=64), causal mask.
Sharding: data-parallel over batch (4) x tensor-parallel over heads (2 groups
of 8). Core c handles batch c//2, head-group c%2. Each core computes its
partial output projection (ctx_g @ Wo_g); the host sums the two head-group
partials per batch and adds the bias.

v3: bf16 + row-packed S + global software pipeline.
- All matmul operands are bf16 (PSUM accumulation stays fp32): same PE
  column rate as fp32r but FWL halves LDWEIGHTS, DMA/SBUF halve, and DVE
  runs at 2x for 16-bit. x is transposed and cast on the host, so the
  on-chip transpose prologue is gone.
- S^T matmuls contract over dh=64 only: head h0 runs on PE rows 0-63 and
  h1 on rows 64-127 (tile_position auto-derived from base_partition).
  Issued adjacently they execute concurrently -> S cost halves.
- One software-pipelined main loop over (chunk j, head pair hp): per
  2-k-tile group, S-quad -> exp (ACT, scale=1/8 folded) -> tril-masked
  PV-quad one group behind. QKV projections for later chunks and the
  output projection of finished chunks are dribbled between groups so the
  PE stays dense while ACT exp hides underneath.
- Causal width restriction on diagonal S/PV matmuls; softmax denominator
  via an appended ones-column of V (PSUM row 64), normalized with DVE
  reciprocal + gpsimd partition_broadcast.
"""

import numpy as np
import ml_dtypes
from contextlib import ExitStack

import concourse.bass as bass
import concourse.tile as tile
from concourse import bacc, mybir

F32 = mybir.dt.float32
BF16 = mybir.dt.bfloat16
FP8 = mybir.dt.float8e4
AF = mybir.ActivationFunctionType
DR = mybir.MatmulPerfMode.DoubleRow
WSC = 32.0                 # fp8 Wq/Wk scale (folded into the exp scale)

B, L, D = 4, 2048, 1024
N_HEAD, DH, DV = 16, 64, 64
N_CORES = 8
HPC = N_HEAD // 2          # heads per core (8)
OC = HPC * DH              # per-core projection width (512)
NHP = HPC // 2             # head-pairs per core (4)
NCH = L // 512             # q-chunks (4)
NLT = L // 128             # l-tiles (16)

BF = ml_dtypes.bfloat16
F8 = ml_dtypes.float8_e4m3fn


def build_nc():
    nc = bacc.Bacc("TRN2", target_bir_lowering=False, debug=False,
                   num_devices=N_CORES)

    xt_d = nc.dram_tensor("xt", [D, L], BF16, kind="ExternalInput").ap()
    wq_d = nc.dram_tensor("wq", [D, OC], BF16, kind="ExternalInput").ap()
    wk_d = nc.dram_tensor("wk", [D, OC], BF16, kind="ExternalInput").ap()
    wv_d = nc.dram_tensor("wv", [D, OC], BF16, kind="ExternalInput").ap()
    wo_d = nc.dram_tensor("wo", [OC, D], BF16, kind="ExternalInput").ap()
    out_d = nc.dram_tensor("out", [L, D], F32, kind="ExternalOutput").ap()

    with tile.TileContext(nc) as tc, ExitStack() as ctx:
        top = ctx.enter_context(tc.tile_pool(name="top", bufs=1))
        psS = ctx.enter_context(tc.tile_pool(name="psS", bufs=2, space="PSUM"))
        psC = ctx.enter_context(tc.tile_pool(name="psC", bufs=2, space="PSUM"))
        psP = ctx.enter_context(tc.tile_pool(name="psP", bufs=2, space="PSUM"))
        pex = ctx.enter_context(tc.tile_pool(name="pex", bufs=4))
        nrm = ctx.enter_context(tc.tile_pool(name="nrm", bufs=3))
        osp = ctx.enter_context(tc.tile_pool(name="osp", bufs=3))

        # per-chunk x^T tiles so early matmuls only wait on their own DMA
        xts = [top.tile([128, 8, 512], BF16, name=f"xt{c}")
               for c in range(NCH)]
        wq_sb = top.tile([128, 8, OC], BF16)
        wk_sb = top.tile([128, 8, OC], BF16)
        wv_sb = top.tile([128, 8, OC], BF16)
        wo_sb = top.tile([128, 4, D], BF16)
        qt = top.tile([128, NHP, L], BF16)        # Q^T per head-pair
        kt = top.tile([128, NHP, L], BF16)        # K^T per head-pair
        vt = top.tile([128, NLT, HPC, DV + 1], BF16)  # V rows + ones col
        ct = top.tile([128, NHP, L], BF16)        # normalized ctx^T
        tril = top.tile([128, 128], F32)
        tril2 = top.tile([128, 2, 128], F32)      # per-head copy for 1-inst mask
        ones = top.tile([128, 1], BF16)

        # ---- input DMAs (chunked xt so proj(0) starts early) ----
        # first-needed first: the serial DMA stream is bandwidth-bound, so
        # order determines when the first matmul (V proj, chunk 0) can start
        nc.sync.dma_start(out=wv_sb,
                          in_=wv_d.rearrange("(t p) o -> p t o", p=128))
        nc.sync.dma_start(
            out=xts[0],
            in_=xt_d[:, 0:512].rearrange("(t p) l -> p t l", p=128))
        nc.sync.dma_start(out=wq_sb,
                          in_=wq_d.rearrange("(t p) o -> p t o", p=128))
        nc.sync.dma_start(out=wk_sb,
                          in_=wk_d.rearrange("(t p) o -> p t o", p=128))
        for c in range(1, NCH):
            nc.sync.dma_start(
                out=xts[c],
                in_=xt_d[:, c * 512:(c + 1) * 512]
                .rearrange("(t p) l -> p t l", p=128))
        nc.sync.dma_start(out=wo_sb,
                          in_=wo_d.rearrange("(v p) o -> p v o", p=128))

        nc.vector.memset(ones, 1.0)
        nc.vector.tensor_copy(
            vt[:, :, :, DV:DV + 1].rearrange("p t h c -> p (t h) c"),
            ones.broadcast_to((128, NLT * HPC, 1)))
        # causal keep-mask for S^T diag blocks: tril[k, q] = 1.0 iff q >= k
        nc.gpsimd.memset(tril, 0.0)
        nc.gpsimd.affine_select(
            out=tril, in_=tril, compare_op=mybir.AluOpType.is_gt,
            fill=1.0, base=0, pattern=[[-1, 128]], channel_multiplier=1)
        for h2 in range(2):
            nc.vector.tensor_copy(tril2[:, h2, :], tril)

        # ---------------- dribbled PE work units ----------------
        def qk_unit(c, hp, w_sb, dst):
            def emit():
                pp = psP.tile([128, 512], F32, tag="pp", name="ppq")
                for d in range(8):
                    nc.tensor.matmul(pp, w_sb[:, d, hp * 128:(hp + 1) * 128],
                                     xts[c][:, d, :],
                                     start=(d == 0), stop=(d == 7))
                nc.vector.tensor_copy(dst[:, hp, c * 512:(c + 1) * 512], pp)
            return emit

        def v_unit(lt):
            def emit():
                pp = psP.tile([128, 512], F32, tag="pp", name="ppv")
                lo = (lt % 4) * 128
                for d in range(8):
                    nc.tensor.matmul(
                        pp, xts[lt // 4][:, d, lo:lo + 128],
                        wv_sb[:, d, :], start=(d == 0), stop=(d == 7))
                nc.vector.tensor_copy(
                    vt[:, lt, :, 0:DV],
                    pp.rearrange("p (h v) -> p h v", h=HPC))
            return emit

        def out_unit(lt, n, state):
            def emit():
                if n == 0:
                    state["ost"] = osp.tile([128, D], F32, tag="ost",
                                            name="ost")
                ost = state["ost"]
                pp = psP.tile([128, 512], F32, tag="pp", name="ppo")
                for v in range(4):
                    nc.tensor.matmul(
                        pp, ct[:, v, lt * 128:(lt + 1) * 128],
                        wo_sb[:, v, n * 512:(n + 1) * 512],
                        start=(v == 0), stop=(v == 3))
                nc.vector.tensor_copy(ost[:, n * 512:(n + 1) * 512], pp)
                if n == 1:
                    nc.sync.dma_start(
                        out=out_d[lt * 128:(lt + 1) * 128, :], in_=ost)
            return emit

        # queue of (key, emit) in dependency order; pump from the front
        queue = []

        def push_qkv(c):
            for hp in range(NHP):
                queue.append((("qk", c, hp), qk_unit(c, hp, wq_sb, qt)))
                queue.append((("qk", c, hp), qk_unit(c, hp, wk_sb, kt)))

        def push_v(c):
            for lt in range(c * 4, c * 4 + 4):
                queue.append((("v", c), v_unit(lt)))

        def push_out(c):
            for lt in range(c * 4, c * 4 + 4):
                state = {}
                for n in range(2):
                    queue.append((("out", c), out_unit(lt, n, state)))

        def pump(n=1):
            for _ in range(n):
                if queue:
                    queue.pop(0)[1]()

        def drain_until(pred):
            """Emit queue units until pred(key) has been emitted for none
            remaining (i.e. pop while any queued unit matches pred)."""
            while any(pred(k) for k, _ in queue):
                pump(1)

        # ---------------- attention unit ----------------
        UNIT_NS = 1700.0       # approx PE ns per dribbled unit

        class Pumper:
            def __init__(self):
                self.acc = 0.0
                self.groups_left = sum(
                    (2 * (j + 1) + 2) for j in range(NCH)) * NHP

            def group(self, need_ns):
                self.acc += need_ns
                self.groups_left -= 1
                while queue and (
                        self.acc >= UNIT_NS
                        or len(queue) > self.groups_left):
                    pump(1)
                    self.acc = max(0.0, self.acc - UNIT_NS)

        pumper = Pumper()

        def attention(j, hp):
            G = 2 * (j + 1)            # 2-ktile groups
            kt_last = 4 * j + 3
            pctx = {}
            for h2 in range(2):
                pctx[h2] = psC.tile([DV + 1, 512], F32, tag="pctx",
                                    name=f"pctx{h2}")
            prevs = [None, None]       # PV lags S by 2 groups
            for g in range(G + 2):
                cur = None
                if g < G:
                    # pexp pair tile: [ktile-in-pair, head, q]
                    cur = pex.tile([128, 2, 2, 512], BF16, tag="pexp",
                                   name="pexp")
                    for r2 in range(2):
                        kt_i = 2 * g + r2
                        c0 = max(0, (kt_i - 4 * j)) * 128
                        # S pair: h0 on PE rows 0-63, h1 on 64-127 (packed)
                        psc = psS.tile([128, 2, 512], F32, tag="psc",
                                       name="psc")
                        for h2 in range(2):
                            po = 64 * h2
                            nc.tensor.matmul(
                                psc[:, h2, c0:512],
                                kt[po:po + DH, hp,
                                   kt_i * 128:(kt_i + 1) * 128],
                                qt[po:po + DH, hp,
                                   j * 512 + c0:(j + 1) * 512],
                                start=True, stop=True)
                        # one exp per ktile covers both heads, exact width
                        nc.scalar.activation(cur[:, r2, :, c0:512],
                                             psc[:, :, c0:512],
                                             AF.Exp, scale=0.125)
                        r = kt_i - 4 * j
                        if 0 <= r < 4:
                            nc.vector.tensor_mul(
                                cur[:, r2, :, r * 128:(r + 1) * 128],
                                cur[:, r2, :, r * 128:(r + 1) * 128],
                                tril2)
                if prevs[0] is not None:
                    pg, pes = prevs[0]
                    for h2 in range(2):
                        for r2 in range(2):
                            kt_i = 2 * pg + r2
                            c0 = max(0, (kt_i - 4 * j)) * 128
                            nc.tensor.matmul(
                                pctx[h2][:, c0:512],
                                vt[:, kt_i, 2 * hp + h2, :],
                                pes[:, r2, h2, c0:512],
                                start=(kt_i == 0), stop=(kt_i == kt_last))
                prevs = [prevs[1], (g, cur) if g < G else None]
                if g >= G:
                    need = 300.0
                elif 2 * g >= 4 * j + 2:
                    need = 500.0
                elif 2 * g >= 4 * j:
                    need = 1000.0
                else:
                    need = 700.0
                pumper.group(need)
            # --- normalize into ct; the two heads' chains are interleaved so
            # the serial gpsimd broadcasts overlap the DVE work
            bcs = {}
            for h2 in range(2):
                rs = nrm.tile([1, 512], F32, tag="rs", name="rs")
                nc.vector.tensor_copy(rs, pctx[h2][DV:DV + 1, :])
                inv = nrm.tile([1, 512], F32, tag="inv", name="inv")
                nc.vector.reciprocal_approx_fast(out=inv, in_=rs)
                bc = nrm.tile([64, 512], F32, tag="bc", name="bc")
                nc.gpsimd.partition_broadcast(out_ap=bc, in_ap=inv)
                bcs[h2] = bc
            for h2 in range(2):
                po = 64 * h2
                nc.vector.tensor_mul(
                    ct[po:po + DV, hp, j * 512:(j + 1) * 512],
                    pctx[h2][0:DV, :], bcs[h2])

        # ---------------- schedule ----------------
        # dense prologue: V(0), Q/K(0, hp0)
        for lt in range(4):
            v_unit(lt)()
        qk_unit(0, 0, wq_sb, qt)()
        qk_unit(0, 0, wk_sb, kt)()
        push_qkv(0)        # hp 1..3 still queued (hp0 re-push skipped below)
        queue[:] = [u for u in queue if u[0] != ("qk", 0, 0)]

        # exp-heavy (3,*) units interleaved with chunk-2 units so out-proj
        # dribble covers them; (0,3) saved for late so out(0) covers (3,3)
        order = [(0, 0), (0, 1), (0, 2),
                 (1, 0), (1, 1), (1, 2), (1, 3),
                 (2, 0), (2, 1), (3, 0), (2, 2),
                 (3, 1), (2, 3), (3, 2), (0, 3), (3, 3)]
        pushed_v = {0}
        pushed_qkv = {0}
        pushed_out = set()

        for idx, (j, hp) in enumerate(order):
            # push future work as soon as its dependencies are scheduled
            if j + 1 < NCH and j + 1 not in pushed_v:
                pushed_v.add(j + 1)
                pushed_qkv.add(j + 1)
                push_v(j + 1)
                push_qkv(j + 1)
            done_chunks = {c for c in range(NCH)
                           if all((jj, hh) in order[:idx]
                                  for hh in range(NHP)
                                  for jj in [c])}
            for c in sorted(done_chunks):
                if c not in pushed_out:
                    pushed_out.add(c)
                    push_out(c)
            # attention(j, hp) requires V(<=j), Q/K(<=j, hp) emitted
            drain_until(lambda k: (k[0] == "v" and k[1] <= j)
                        or (k[0] == "qk" and k[1] <= j and k[2] == hp))
            attention(j, hp)

        # drain the remainder (out-proj of chunk 3 + leftovers)
        push_out(3)
        while queue:
            pump(1)

    nc.compile()
    return nc


def make_in_maps(x, Wq, Wk, Wv, Wo):
    in_maps = []
    for c in range(N_CORES):
        b, g = c // 2, c % 2
        in_maps.append({
            "xt": np.ascontiguousarray(x[b].T).astype(BF),
            "wq": np.ascontiguousarray(Wq[:, g * OC:(g + 1) * OC]).astype(BF),
            "wk": np.ascontiguousarray(Wk[:, g * OC:(g + 1) * OC]).astype(BF),
            "wv": np.ascontiguousarray(Wv[:, g * OC:(g + 1) * OC]).astype(BF),
            "wo": np.ascontiguousarray(Wo[g * OC:(g + 1) * OC, :]).astype(BF),
        })
    return in_maps


_NC_CACHE = {}


def _get_nc():
    if "nc" not in _NC_CACHE:
        _NC_CACHE["nc"] = build_nc()
    return _NC_CACHE["nc"]


def _numpy_fallback(x, Wq, Wk, Wv, Wo, bo, mask):
    Bsz, Lq, _ = x.shape
    Q = (x @ Wq).reshape(Bsz, Lq, N_HEAD, DH).transpose(0, 2, 1, 3)
    K = (x @ Wk).reshape(Bsz, Lq, N_HEAD, DH).transpose(0, 2, 1, 3)
    V = (x @ Wv).reshape(Bsz, Lq, N_HEAD, DV).transpose(0, 2, 1, 3)
    s = np.einsum("bhqd,bhkd->bhqk", Q, K) / np.sqrt(np.float32(DH))
    s = np.where(mask, s, -np.inf)
    s = s - s.max(axis=-1, keepdims=True)
    p = np.exp(s)
    p /= p.sum(axis=-1, keepdims=True)
    ctxv = np.einsum("bhqk,bhkv->bhqv", p, V)
    ctxv = ctxv.transpose(0, 2, 1, 3).reshape(Bsz, Lq, N_HEAD * DV)
    return (ctxv @ Wo + bo).astype(np.float32)


def run_on_hw(in_maps, trace=False):
    from concourse.bass_utils import run_bass_kernel_spmd
    nc = _get_nc()
    return run_bass_kernel_spmd(nc, in_maps, list(range(N_CORES)), trace=trace)


def kernel(x, Wq, Wk, Wv, Wo, bo, mask, _trace=False, _results=None):
    x = np.asarray(x, dtype=np.float32)
    Wq = np.asarray(Wq, dtype=np.float32)
    Wk = np.asarray(Wk, dtype=np.float32)
    Wv = np.asarray(Wv, dtype=np.float32)
    Wo = np.asarray(Wo, dtype=np.float32)
    bo = np.asarray(bo, dtype=np.float32)
    mask_np = np.asarray(mask).reshape(mask.shape[-2], mask.shape[-1])

    causal = bool(np.array_equal(
        mask_np, np.tril(np.ones((L, L), dtype=bool))))
    if not causal or x.shape != (B, L, D):
        return _numpy_fallback(np.asarray(x), Wq, Wk, Wv, Wo, bo,
                               np.asarray(mask))

    res = run_on_hw(make_in_maps(x, Wq, Wk, Wv, Wo), trace=_trace)
    if _results is not None:
        _results.append(res)
    out = np.empty((B, L, D), dtype=np.float32)
    for b in range(B):
        out[b] = res.results[2 * b]["out"] + res.results[2 * b + 1]["out"] + bo
    return out


# revision 44
# speedup vs baseline: 1.0021x; 1.0021x over previous
"""Causal multi-head attention Trainium2 kernel (8 NeuronCores).

Problem: B=4, L=2048, D=1024, 16 heads x (dh=64, dv=64), causal mask.
Sharding: data-parallel over batch (4) x tensor-parallel over heads (2 groups
of 8). Core c handles batch c//2, head-group c%2. Each core computes its
partial output projection (ctx_g @ Wo_g); the host sums the two head-group
partials per batch and adds the bias.

v4: bf16 + row-packed S + global software pipeline (~281 us, 1.53x over
the fp32r v2 baseline's 430 us).
- All matmul operands are bf16 (PSUM accumulation stays fp32): same PE
  column rate as fp32r but FWL halves LDWEIGHTS, DMA/SBUF halve, and DVE
  runs at 2x for 16-bit. x is transposed and cast on the host, so the
  on-chip transpose prologue is gone.
- S^T matmuls contract over dh=64 only: head h0 runs on PE rows 0-63 and
  h1 on rows 64-127 (tile_position auto-derived from base_partition).
  Both heads' S writes share one psc tile freed by a single exp, so they
  become ready together and the PE runs them concurrently (row packing).
- One software-pipelined main loop over (chunk j, head pair hp): per
  2-k-tile group, S-quad -> one exp per k-tile (exact causal width, both
  heads) -> tril-masked PV-quad two groups behind. QKV projections for
  later chunks and the output projection of finished chunks are dribbled
  between groups so the PE stays dense while ACT exp hides underneath;
  exp-heavy (3,*) attention units are interleaved with chunk-2 units and
  (0,3) is saved for late so out-proj dribble covers (3,3).
- Causal width restriction on diagonal S/PV matmuls; softmax denominator
  via an appended ones-column of V (PSUM row 64), normalized with DVE
  reciprocal + gpsimd partition_broadcast.
- Input DMAs ordered first-needed-first (wv, xt chunk 0, wq, wk, ...);
  per-chunk xt tiles so early matmuls only wait on their own DMA.

Measured rejects: fp8 DoubleRow projections (1.44x PE rate on paper) and
a K=1-matmul broadcast both push the chip into a sustained ~20% clock
throttle (P0) - net losses. V/pexp in fp8 fail the 2e-2 gate outright.
PSUM's 8 banks pin the pipeline shape: psc 2x[128,2,512] (4) + pctx
2x[65,512] (2) + proj/out pp 2x[128,512] (2).
"""

import numpy as np
import ml_dtypes
from contextlib import ExitStack

import concourse.bass as bass
import concourse.tile as tile
from concourse import bacc, mybir

F32 = mybir.dt.float32
BF16 = mybir.dt.bfloat16
FP8 = mybir.dt.float8e4
AF = mybir.ActivationFunctionType
DR = mybir.MatmulPerfMode.DoubleRow
WSC = 32.0                 # fp8 Wq/Wk scale (folded into the exp scale)

B, L, D = 4, 2048, 1024
N_HEAD, DH, DV = 16, 64, 64
N_CORES = 8
HPC = N_HEAD // 2          # heads per core (8)
OC = HPC * DH              # per-core projection width (512)
NHP = HPC // 2             # head-pairs per core (4)
NCH = L // 512             # q-chunks (4)
NLT = L // 128             # l-tiles (16)

BF = ml_dtypes.bfloat16
F8 = ml_dtypes.float8_e4m3fn


def build_nc():
    nc = bacc.Bacc("TRN2", target_bir_lowering=False, debug=False,
                   num_devices=N_CORES)

    xt_d = nc.dram_tensor("xt", [D, L], BF16, kind="ExternalInput").ap()
    wq_d = nc.dram_tensor("wq", [D, OC], BF16, kind="ExternalInput").ap()
    wk_d = nc.dram_tensor("wk", [D, OC], BF16, kind="ExternalInput").ap()
    wv_d = nc.dram_tensor("wv", [D, OC], BF16, kind="ExternalInput").ap()
    wo_d = nc.dram_tensor("wo", [OC, D], BF16, kind="ExternalInput").ap()
    out_d = nc.dram_tensor("out", [L, D], F32, kind="ExternalOutput").ap()

    with tile.TileContext(nc) as tc, ExitStack() as ctx:
        top = ctx.enter_context(tc.tile_pool(name="top", bufs=1))
        psS = ctx.enter_context(tc.tile_pool(name="psS", bufs=2, space="PSUM"))
        psC = ctx.enter_context(tc.tile_pool(name="psC", bufs=2, space="PSUM"))
        psP = ctx.enter_context(tc.tile_pool(name="psP", bufs=2, space="PSUM"))
        pex = ctx.enter_context(tc.tile_pool(name="pex", bufs=4))
        nrm = ctx.enter_context(tc.tile_pool(name="nrm", bufs=3))
        osp = ctx.enter_context(tc.tile_pool(name="osp", bufs=3))

        # per-chunk x^T tiles so early matmuls only wait on their own DMA
        xts = [top.tile([128, 8, 512], BF16, name=f"xt{c}")
               for c in range(NCH)]
        wq_sb = top.tile([128, 8, OC], BF16)
        wk_sb = top.tile([128, 8, OC], BF16)
        wv_sb = top.tile([128, 8, OC], BF16)
        wo_sb = top.tile([128, 4, D], BF16)
        qt = top.tile([128, NHP, L], BF16)        # Q^T per head-pair
        kt = top.tile([128, NHP, L], BF16)        # K^T per head-pair
        vt = top.tile([128, NLT, HPC, DV + 1], BF16)  # V rows + ones col
        ct = top.tile([128, NHP, L], BF16)        # normalized ctx^T
        tril = top.tile([128, 128], F32)
        ones = top.tile([128, 1], BF16)

        # ---- input DMAs (chunked xt so proj(0) starts early) ----
        # first-needed first: the serial DMA stream is bandwidth-bound, so
        # order determines when the first matmul (V proj, chunk 0) can start
        nc.sync.dma_start(out=wv_sb,
                          in_=wv_d.rearrange("(t p) o -> p t o", p=128))
        nc.sync.dma_start(
            out=xts[0],
            in_=xt_d[:, 0:512].rearrange("(t p) l -> p t l", p=128))
        nc.sync.dma_start(out=wq_sb,
                          in_=wq_d.rearrange("(t p) o -> p t o", p=128))
        nc.sync.dma_start(out=wk_sb,
                          in_=wk_d.rearrange("(t p) o -> p t o", p=128))
        for c in range(1, NCH):
            nc.sync.dma_start(
                out=xts[c],
                in_=xt_d[:, c * 512:(c + 1) * 512]
                .rearrange("(t p) l -> p t l", p=128))
        nc.sync.dma_start(out=wo_sb,
                          in_=wo_d.rearrange("(v p) o -> p v o", p=128))

        nc.vector.memset(ones, 1.0)
        nc.vector.tensor_copy(
            vt[:, :, :, DV:DV + 1].rearrange("p t h c -> p (t h) c"),
            ones.broadcast_to((128, NLT * HPC, 1)))
        # causal keep-mask for S^T diag blocks: tril[k, q] = 1.0 iff q >= k
        nc.gpsimd.memset(tril, 0.0)
        nc.gpsimd.affine_select(
            out=tril, in_=tril, compare_op=mybir.AluOpType.is_gt,
            fill=1.0, base=0, pattern=[[-1, 128]], channel_multiplier=1)

        # ---------------- dribbled PE work units ----------------
        def qk_unit(c, hp, w_sb, dst):
            def emit():
                pp = psP.tile([128, 512], F32, tag="pp", name="ppq")
                for d in range(8):
                    nc.tensor.matmul(pp, w_sb[:, d, hp * 128:(hp + 1) * 128],
                                     xts[c][:, d, :],
                                     start=(d == 0), stop=(d == 7))
                nc.vector.tensor_copy(dst[:, hp, c * 512:(c + 1) * 512], pp)
            return emit

        def v_unit(lt):
            def emit():
                pp = psP.tile([128, 512], F32, tag="pp", name="ppv")
                lo = (lt % 4) * 128
                for d in range(8):
                    nc.tensor.matmul(
                        pp, xts[lt // 4][:, d, lo:lo + 128],
                        wv_sb[:, d, :], start=(d == 0), stop=(d == 7))
                nc.vector.tensor_copy(
                    vt[:, lt, :, 0:DV],
                    pp.rearrange("p (h v) -> p h v", h=HPC))
            return emit

        def out_unit(lt, n, state):
            def emit():
                if n == 0:
                    state["ost"] = osp.tile([128, D], F32, tag="ost",
                                            name="ost")
                ost = state["ost"]
                pp = psP.tile([128, 512], F32, tag="pp", name="ppo")
                for v in range(4):
                    nc.tensor.matmul(
                        pp, ct[:, v, lt * 128:(lt + 1) * 128],
                        wo_sb[:, v, n * 512:(n + 1) * 512],
                        start=(v == 0), stop=(v == 3))
                nc.vector.tensor_copy(ost[:, n * 512:(n + 1) * 512], pp)
                if n == 1:
                    nc.sync.dma_start(
                        out=out_d[lt * 128:(lt + 1) * 128, :], in_=ost)
            return emit

        # queue of (key, emit) in dependency order; pump from the front
        queue = []

        def push_qkv(c):
            for hp in range(NHP):
                queue.append((("qk", c, hp), qk_unit(c, hp, wq_sb, qt)))
                queue.append((("qk", c, hp), qk_unit(c, hp, wk_sb, kt)))

        def push_v(c):
            for lt in range(c * 4, c * 4 + 4):
                queue.append((("v", c), v_unit(lt)))

        def push_out(c):
            for lt in range(c * 4, c * 4 + 4):
                state = {}
                for n in range(2):
                    queue.append((("out", c), out_unit(lt, n, state)))

        def pump(n=1):
            for _ in range(n):
                if queue:
                    queue.pop(0)[1]()

        def drain_until(pred):
            """Emit queue units until pred(key) has been emitted for none
            remaining (i.e. pop while any queued unit matches pred)."""
            while any(pred(k) for k, _ in queue):
                pump(1)

        # ---------------- attention unit ----------------
        UNIT_NS = 1700.0       # approx PE ns per dribbled unit

        class Pumper:
            def __init__(self):
                self.acc = 0.0
                self.groups_left = sum(
                    (2 * (j + 1) + 2) for j in range(NCH)) * NHP

            def group(self, need_ns):
                self.acc += need_ns
                self.groups_left -= 1
                while queue and (
                        self.acc >= UNIT_NS
                        or len(queue) > self.groups_left):
                    pump(1)
                    self.acc = max(0.0, self.acc - UNIT_NS)

        pumper = Pumper()

        def attention(j, hp):
            G = 2 * (j + 1)            # 2-ktile groups
            kt_last = 4 * j + 3
            pctx = {}
            for h2 in range(2):
                pctx[h2] = psC.tile([DV + 1, 512], F32, tag="pctx",
                                    name=f"pctx{h2}")
            prevs = [None, None]       # PV lags S by 2 groups
            for g in range(G + 2):
                cur = None
                if g < G:
                    # pexp pair tile: [ktile-in-pair, head, q]
                    cur = pex.tile([128, 2, 2, 512], BF16, tag="pexp",
                                   name="pexp")
                    for r2 in range(2):
                        kt_i = 2 * g + r2
                        c0 = max(0, (kt_i - 4 * j)) * 128
                        # S pair: h0 on PE rows 0-63, h1 on 64-127 (packed)
                        psc = psS.tile([128, 2, 512], F32, tag="psc",
                                       name="psc")
                        for h2 in range(2):
                            po = 64 * h2
                            nc.tensor.matmul(
                                psc[:, h2, c0:512],
                                kt[po:po + DH, hp,
                                   kt_i * 128:(kt_i + 1) * 128],
                                qt[po:po + DH, hp,
                                   j * 512 + c0:(j + 1) * 512],
                                start=True, stop=True)
                        # one exp per ktile covers both heads, exact width
                        nc.scalar.activation(cur[:, r2, :, c0:512],
                                             psc[:, :, c0:512],
                                             AF.Exp, scale=0.125)
                        r = kt_i - 4 * j
                        if 0 <= r < 4:
                            for h2 in range(2):
                                nc.vector.tensor_mul(
                                    cur[:, r2, h2, r * 128:(r + 1) * 128],
                                    cur[:, r2, h2, r * 128:(r + 1) * 128],
                                    tril)
                if prevs[0] is not None:
                    pg, pes = prevs[0]
                    for h2 in range(2):
                        for r2 in range(2):
                            kt_i = 2 * pg + r2
                            c0 = max(0, (kt_i - 4 * j)) * 128
                            nc.tensor.matmul(
                                pctx[h2][:, c0:512],
                                vt[:, kt_i, 2 * hp + h2, :],
                                pes[:, r2, h2, c0:512],
                                start=(kt_i == 0), stop=(kt_i == kt_last))
                prevs = [prevs[1], (g, cur) if g < G else None]
                if g >= G:
                    need = 300.0
                elif 2 * g >= 4 * j + 2:
                    need = 500.0
                elif 2 * g >= 4 * j:
                    need = 1000.0
                else:
                    need = 700.0
                pumper.group(need)
            # --- normalize into ct
            for h2 in range(2):
                po = 64 * h2
                rs = nrm.tile([1, 512], F32, tag="rs", name="rs")
                nc.vector.tensor_copy(rs, pctx[h2][DV:DV + 1, :])
                inv = nrm.tile([1, 512], F32, tag="inv", name="inv")
                nc.vector.reciprocal_approx_fast(out=inv, in_=rs)
                bc = nrm.tile([64, 512], F32, tag="bc", name="bc")
                nc.gpsimd.partition_broadcast(out_ap=bc, in_ap=inv)
                nc.vector.tensor_mul(
                    ct[po:po + DV, hp, j * 512:(j + 1) * 512],
                    pctx[h2][0:DV, :], bc)

        # ---------------- schedule ----------------
        # dense prologue: V(0), Q/K(0, hp0)
        for lt in range(4):
            v_unit(lt)()
        qk_unit(0, 0, wq_sb, qt)()
        qk_unit(0, 0, wk_sb, kt)()
        push_qkv(0)        # hp 1..3 still queued (hp0 re-push skipped below)
        queue[:] = [u for u in queue if u[0] != ("qk", 0, 0)]

        # exp-heavy (3,*) units interleaved with chunk-2 units so out-proj
        # dribble covers them; (0,3) saved for late so out(0) covers (3,3)
        order = [(0, 0), (0, 1), (0, 2),
                 (1, 0), (1, 1), (1, 2), (1, 3),
                 (2, 0), (2, 1), (3, 0), (2, 2),
                 (3, 1), (2, 3), (3, 2), (0, 3), (3, 3)]
        pushed_v = {0}
        pushed_qkv = {0}
        pushed_out = set()

        for idx, (j, hp) in enumerate(order):
            # push future work as soon as its dependencies are scheduled
            if j + 1 < NCH and j + 1 not in pushed_v:
                pushed_v.add(j + 1)
                pushed_qkv.add(j + 1)
                push_v(j + 1)
                push_qkv(j + 1)
            done_chunks = {c for c in range(NCH)
                           if all((jj, hh) in order[:idx]
                                  for hh in range(NHP)
                                  for jj in [c])}
            for c in sorted(done_chunks):
                if c not in pushed_out:
                    pushed_out.add(c)
                    push_out(c)
            # attention(j, hp) requires V(<=j), Q/K(<=j, hp) emitted
            drain_until(lambda k: (k[0] == "v" and k[1] <= j)
                        or (k[0] == "qk" and k[1] <= j and k[2] == hp))
            attention(j, hp)

        # drain the remainder (out-proj of chunk 3 + leftovers)
        push_out(3)
        while queue:
            pump(1)

    nc.compile()
    return nc


def make_in_maps(x, Wq, Wk, Wv, Wo):
    in_maps = []
    for c in range(N_CORES):
        b, g = c // 2, c % 2
        in_maps.append({
            "xt": np.ascontiguousarray(x[b].T).astype(BF),
            "wq": np.ascontiguousarray(Wq[:, g * OC:(g + 1) * OC]).astype(BF),
            "wk": np.ascontiguousarray(Wk[:, g * OC:(g + 1) * OC]).astype(BF),
            "wv": np.ascontiguousarray(Wv[:, g * OC:(g + 1) * OC]).astype(BF),
            "wo": np.ascontiguousarray(Wo[g * OC:(g + 1) * OC, :]).astype(BF),
        })
    return in_maps


_NC_CACHE = {}


def _get_nc():
    if "nc" not in _NC_CACHE:
        _NC_CACHE["nc"] = build_nc()
    return _NC_CACHE["nc"]


def _numpy_fallback(x, Wq, Wk, Wv, Wo, bo, mask):
    Bsz, Lq, _ = x.shape
    Q = (x @ Wq).reshape(Bsz, Lq, N_HEAD, DH).transpose(0, 2, 1, 3)
    K = (x @ Wk).reshape(Bsz, Lq, N_HEAD, DH).transpose(0, 2, 1, 3)
    V = (x @ Wv).reshape(Bsz, Lq, N_HEAD, DV).transpose(0, 2, 1, 3)
    s = np.einsum("bhqd,bhkd->bhqk", Q, K) / np.sqrt(np.float32(DH))
    s = np.where(mask, s, -np.inf)
    s = s - s.max(axis=-1, keepdims=True)
    p = np.exp(s)
    p /= p.sum(axis=-1, keepdims=True)
    ctxv = np.einsum("bhqk,bhkv->bhqv", p, V)
    ctxv = ctxv.transpose(0, 2, 1, 3).reshape(Bsz, Lq, N_HEAD * DV)
    return (ctxv @ Wo + bo).astype(np.float32)


def run_on_hw(in_maps, trace=False):
    from concourse.bass_utils import run_bass_kernel_spmd
    nc = _get_nc()
    return run_bass_kernel_spmd(nc, in_maps, list(range(N_CORES)), trace=trace)


def kernel(x, Wq, Wk, Wv, Wo, bo, mask, _trace=False, _results=None):
    x = np.asarray(x, dtype=np.float32)
    Wq = np.asarray(Wq, dtype=np.float32)
    Wk = np.asarray(Wk, dtype=np.float32)
    Wv = np.asarray(Wv, dtype=np.float32)
    Wo = np.asarray(Wo, dtype=np.float32)
    bo = np.asarray(bo, dtype=np.float32)
    mask_np = np.asarray(mask).reshape(mask.shape[-2], mask.shape[-1])

    causal = bool(np.array_equal(
        mask_np, np.tril(np.ones((L, L), dtype=bool))))
    if not causal or x.shape != (B, L, D):
        return _numpy_fallback(np.asarray(x), Wq, Wk, Wv, Wo, bo,
                               np.asarray(mask))

    res = run_on_hw(make_in_maps(x, Wq, Wk, Wv, Wo), trace=_trace)
    if _results is not None:
        _results.append(res)
    out = np.empty((B, L, D), dtype=np.float32)
    for b in range(B):
        out[b] = res.results[2 * b]["out"] + res.results[2 * b + 1]["out"] + bo
    return out
